# revision 17
# baseline (speedup 1.0000x reference)
"""Trainium2 Bass kernel for nn_LinearCrossAttention.

Math (per batch b, per stream s with "other" stream o):
  q = delu(wq_s x_s + bq_s); k = delu(wk_s x_s + bk_s); v = wv_s x_s + bv_s
  ko = delu(wk_o x_o + bk_o)                      (other-stream k)
  qsum[n]  = sum_c q[c,n]
  esum[c]  = sum_n qsum[n] * ko[c,n]              (== energy_s.sum(axis=1))
  norm[n]  = 1 / (sum_c esum[c] * ko[c,n] + eps)
  att[c,n] = esum[c] * k[c,n] * v[c,n] * norm[n]
  out_s = x_s + gamma_s * (att + wd1 x_s + bd1 + conv3x3(x_s, wd3) + bd3)

delu(y) = 10*relu(y) + expm1(10*min(y,0)) + 1 = relu(10y) + min(exp(10y), 1)

Sharding: 8 cores = 4 batches x 2 streams; fully independent (the other
stream's k is recomputed locally), no collectives.
"""

import numpy as np
from contextlib import ExitStack

# Problem dims (hardcoded per harness contract)
B, C, H, W = 4, 256, 128, 128
EPS = 1e-10
PARAM = 10.0
N_CORES = 8


def build_nc(Hd=H, Wd=W, CR=None):
    """Build the per-core Bass program. Hd/Wd/CR overridable for sim tests."""
    import concourse.bass as bass
    import concourse.mybir as mybir
    import concourse.tile as tile
    from concourse import bacc

    dt = mybir.dt
    f32 = dt.float32
    f32r = dt.float32r
    AF = mybir.ActivationFunctionType
    OP = mybir.AluOpType

    if CR is None:
        CR = max(1, 512 // Wd)  # chunk rows such that F <= 512
    F = CR * Wd                 # matmul moving free size per chunk
    NCH = Hd // CR
    assert Hd % CR == 0
    KT = C // 128               # 2 k-tiles
    MT = C // 128               # 2 m-tiles
    HW = Hd * Wd

    nc = bacc.Bacc()

    xa_d = nc.dram_tensor("xap", (C, Hd + 2, Wd + 2), f32, kind="ExternalInput")
    xb_d = nc.dram_tensor("xb", (C, Hd, Wd), f32, kind="ExternalInput")
    wq_d = nc.dram_tensor("wqT", (C, C), f32, kind="ExternalInput")
    wk_d = nc.dram_tensor("wkT", (C, C), f32, kind="ExternalInput")
    wv_d = nc.dram_tensor("wvT", (C, C), f32, kind="ExternalInput")
    wko_d = nc.dram_tensor("wkoT", (C, C), f32, kind="ExternalInput")
    wd1_d = nc.dram_tensor("wd1T", (C, C), f32, kind="ExternalInput")
    wd3_d = nc.dram_tensor("wd3T", (9, C, C), f32, kind="ExternalInput")
    bq_d = nc.dram_tensor("bq", (1, C), f32, kind="ExternalInput")
    bk_d = nc.dram_tensor("bk", (1, C), f32, kind="ExternalInput")
    bv_d = nc.dram_tensor("bv", (1, C), f32, kind="ExternalInput")
    bko_d = nc.dram_tensor("bko", (1, C), f32, kind="ExternalInput")
    bsum_d = nc.dram_tensor("bsum", (1, C), f32, kind="ExternalInput")
    gam_d = nc.dram_tensor("gam", (1, 1), f32, kind="ExternalInput")
    out_d = nc.dram_tensor("out", (C, Hd, Wd), f32, kind="ExternalOutput")

    # DRAM views: channel dim -> (tile, partition)
    xa_v = xa_d.rearrange("(t p) h w -> p t h w", p=128)  # padded H+2 x W+2
    xb_v = xb_d.rearrange("(t p) h w -> p t (h w)", p=128)
    out_v = out_d.rearrange("(t p) h w -> p t h w", p=128)

    with tile.TileContext(nc) as tc, ExitStack() as ctx, \
            nc.allow_low_precision(reason="fp32r matmul operands (fp22 ok)"):
        const = ctx.enter_context(tc.tile_pool(name="const", bufs=1))
        wpool = ctx.enter_context(tc.tile_pool(name="w", bufs=1))
        io3 = ctx.enter_context(tc.tile_pool(name="io3", bufs=3))
        stage = ctx.enter_context(tc.tile_pool(name="stage", bufs=2))
        outp = ctx.enter_context(tc.tile_pool(name="outp", bufs=3))
        small = ctx.enter_context(tc.tile_pool(name="small", bufs=2))
        psum = ctx.enter_context(
            tc.tile_pool(name="psum", bufs=1, space=bass.MemorySpace.PSUM)
        )

        # ---- constants / weights ----
        ones_row = const.tile([1, F], f32, tag="ones_row", name="ones_row")
        nc.vector.memset(ones_row, 1.0)
        ones_col = const.tile([128, 1], f32, tag="ones_col", name="ones_col")
        nc.vector.memset(ones_col, 1.0)
        ones_r128 = const.tile([1, 128], f32, tag="ones_r128", name="ones_r128")
        nc.vector.memset(ones_r128, 1.0)
        eps_t = const.tile([1, 1], f32, tag="eps", name="eps")
        nc.vector.memset(eps_t, EPS)
        gam_col = const.tile([128, 1], f32, tag="gam_col", name="gam_col")
        nc.sync.dma_start(
            out=gam_col,
            in_=bass.AP(tensor=gam_d, offset=0, ap=[[0, 128], [1, 1]]),
        )

        w_sb = {}
        for name, d in (("wq", wq_d), ("wk", wk_d), ("wv", wv_d),
                        ("wko", wko_d), ("wd1", wd1_d)):
            t = wpool.tile([128, KT, C], f32, tag=name)
            nc.sync.dma_start(out=t.bitcast(f32r), in_=d.rearrange("(t p) o -> p t o", p=128).bitcast(f32r))
            w_sb[name] = t
        wd3_sb = wpool.tile([128, 9 * KT, C], f32, tag="wd3", name="wd3")
        nc.sync.dma_start(
            out=wd3_sb.bitcast(f32r), in_=wd3_d.rearrange("k (t p) o -> p (k t) o", p=128).bitcast(f32r)
        )
        b_sb = {}
        for name, d in (("bq", bq_d), ("bk", bk_d), ("bv", bv_d),
                        ("bko", bko_d), ("bsum", bsum_d)):
            t = const.tile([1, C], f32, tag=name)
            nc.sync.dma_start(out=t.bitcast(f32r), in_=d[:].bitcast(f32r))
            b_sb[name] = t

        esum = const.tile([128, MT], f32, tag="esum", name="esum")
        nc.vector.memset(esum, 0.0)
        esg = const.tile([128, MT], f32, tag="esg", name="esg")


        def pe_touch(ap):
            # tiny bf16 ldweights: absorbs one fresh semaphore onto the PE
            # queue so the following fp32r matmul (1 wait slot) stays legal
            nc.tensor.ldweights(ap.bitcast(dt.bfloat16))

        def mm_proj(ps_list, wname, bname, rhs_fn, start_extra=None):
            """ps[mt] = bias row + W.T @ rhs (bias matmul first)."""
            w = w_sb[wname]
            for mt in range(MT):
                msl = slice(mt * 128, (mt + 1) * 128)
                nc.tensor.matmul(
                    ps_list[mt],
                    b_sb[bname][:, msl].bitcast(f32r),
                    ones_row.bitcast(f32r),
                    start=True,
                    stop=False,
                )
                for kt in range(KT):
                    nc.tensor.matmul(
                        ps_list[mt],
                        w[:, kt, msl].bitcast(f32r),
                        rhs_fn(kt),
                        start=False,
                        stop=(kt == KT - 1),
                    )

        def delu(dst, ps_list, e_t, r_t):
            """dst[:, mt, :] = relu(10*ps) + min(exp(10*ps), 1)."""
            for mt in range(MT):
                nc.scalar.activation(e_t[:, mt, :], ps_list[mt], AF.Exp,
                                     bias=0.0, scale=PARAM)
                nc.scalar.activation(r_t[:, mt, :], ps_list[mt], AF.Relu,
                                     bias=0.0, scale=PARAM)
            nc.vector.scalar_tensor_tensor(
                dst.rearrange("p t f -> p (t f)").bitcast(f32r),
                e_t.rearrange("p t f -> p (t f)"),
                1.0,
                r_t.rearrange("p t f -> p (t f)"),
                op0=OP.min,
                op1=OP.add,
            )

        # ================= PASS 1: qsum / esum =================
        for ci in range(NCH):
            n0 = ci * F
            r0 = ci * CR
            xa_c = io3.tile([128, KT, F], f32, tag="xa1", name="xa1")
            for kt in range(KT):
                nc.sync.dma_start(
                    out=xa_c[:, kt, :].rearrange(
                        "p (a b) -> p a b", a=CR).bitcast(f32r),
                    in_=xa_v[:, kt, r0 + 1:r0 + 1 + CR, 1:Wd + 1].bitcast(f32r),
                )
            xb_c = io3.tile([128, KT, F], f32, tag="xb1", name="xb1")
            nc.sync.dma_start(out=xb_c.bitcast(f32r), in_=xb_v[:, :, n0:n0 + F].bitcast(f32r))

            q_ps = [psum.tile([128, F], f32, tag=f"A{mt}", name=f"A{mt}") for mt in range(MT)]
            mm_proj(q_ps, "wq", "bq", lambda kt: xa_c[:, kt, :].bitcast(f32r))
            e_t = stage.tile([128, MT, F], f32, tag="e", name="e")
            r_t = stage.tile([128, MT, F], f32, tag="r", name="r")
            q_sb = stage.tile([128, MT, F], f32, tag="qk", name="qk")
            delu(q_sb, q_ps, e_t, r_t)

            ko_ps = [psum.tile([128, F], f32, tag=f"B{mt}", name=f"B{mt}") for mt in range(MT)]
            mm_proj(ko_ps, "wko", "bko", lambda kt: xb_c[:, kt, :].bitcast(f32r))
            e2_t = stage.tile([128, MT, F], f32, tag="e", name="e")
            r2_t = stage.tile([128, MT, F], f32, tag="r", name="r")
            ko_sb = stage.tile([128, MT, F], f32, tag="ko", name="ko")
            delu(ko_sb, ko_ps, e2_t, r2_t)

            # qsum (1,F) = column sums of q
            qs_ps = psum.tile([1, F], f32, tag="C0", name="C0")
            for mt in range(MT):
                nc.tensor.matmul(
                    qs_ps,
                    ones_col.bitcast(f32r),
                    q_sb[:, mt, :].bitcast(f32r),
                    start=(mt == 0),
                    stop=(mt == MT - 1),
                )
            qsum_sb = small.tile([1, F], f32, tag="qsum", name="qsum")
            nc.scalar.copy(qsum_sb.bitcast(f32r), qs_ps)
            qb_ps = psum.tile([128, F], f32, tag="C1", name="C1")
            nc.tensor.matmul(
                qb_ps, ones_r128.bitcast(f32r), qsum_sb.bitcast(f32r),
                start=True, stop=True,
            )

            # esum[c] += sum_n ko[c,n] * qsum[n]
            ech = small.tile([128, MT], f32, tag="ech", name="ech")
            scr = stage.tile([128, F], f32, tag="scr", name="scr")
            for mt in range(MT):
                nc.vector.scalar_tensor_tensor(
                    scr, ko_sb[:, mt, :], 1.0, qb_ps,
                    op0=OP.mult, op1=OP.mult,
                    accum_out=ech[:, mt:mt + 1],
                )
            nc.vector.tensor_add(esum.bitcast(f32r), esum, ech)

        # esg = gamma * esum
        nc.vector.tensor_scalar_mul(esg, esum, gam_col)

        # ================= PASS 2: output =================
        Wp = Wd + 2
        for ci in range(NCH):
            r0 = ci * CR
            # halo tiles (per kt): rows r0 .. r0+CR+2 of the padded input
            xah_l = []
            for kt in range(KT):
                xh = io3.tile([128, CR + 2, Wp], f32, tag=f"xah{kt}",
                              name=f"xah{kt}")
                nc.sync.dma_start(
                    out=xh.bitcast(f32r),
                    in_=xa_v[:, kt, r0:r0 + CR + 2, :].bitcast(f32r),
                )
                xah_l.append(xh)
            xb_c = io3.tile([128, KT, F], f32, tag="xb1", name="xb1")
            n0 = ci * F
            nc.sync.dma_start(out=xb_c.bitcast(f32r), in_=xb_v[:, :, n0:n0 + F].bitcast(f32r))
            # separate copy of the xa residual for the DVE combine (keeps the
            # halo tiles PE-only so slot-release waits stay single-engine)
            xar = io3.tile([128, MT, CR, Wd], f32, tag="xar", name="xar")
            for kt in range(KT):
                nc.sync.dma_start(
                    out=xar[:, kt],
                    in_=xa_v[:, kt, r0 + 1:r0 + 1 + CR, 1:Wd + 1],
                )

            def xa_rhs(kt, ky=1, kx=1, _xah=xah_l):
                return _xah[kt][:, ky:ky + CR, kx:kx + Wd].bitcast(f32r)

            # ko (other-stream k) for norm
            ko_ps = [psum.tile([128, F], f32, tag=f"A{mt}", name=f"A{mt}") for mt in range(MT)]
            mm_proj(ko_ps, "wko", "bko", lambda kt: xb_c[:, kt, :].bitcast(f32r))
            e_t = stage.tile([128, MT, F], f32, tag="e", name="e")
            r_t = stage.tile([128, MT, F], f32, tag="r", name="r")
            ko_sb = stage.tile([128, MT, F], f32, tag="ko", name="ko")
            delu(ko_sb, ko_ps, e_t, r_t)

            # norm denominator (1,F) = esum . ko + eps
            nd_ps = psum.tile([1, F], f32, tag="C0", name="C0")
            for mt in range(MT):
                nc.tensor.matmul(
                    nd_ps,
                    esum[:, mt:mt + 1].bitcast(f32r),
                    ko_sb[:, mt, :].bitcast(f32r),
                    start=(mt == 0),
                    stop=False,
                )
            nc.tensor.matmul(
                nd_ps, eps_t.bitcast(f32r), ones_row.bitcast(f32r),
                start=False, stop=True,
            )
            nr_sb = small.tile([1, F], f32, tag="nr", name="nr")
            nc.vector.reciprocal(nr_sb.bitcast(f32r), nd_ps)
            nb_ps = psum.tile([128, F], f32, tag="C1", name="C1")
            nc.tensor.matmul(
                nb_ps, ones_r128.bitcast(f32r), nr_sb.bitcast(f32r),
                start=True, stop=True,
            )

            # k (own stream)
            k_ps = [psum.tile([128, F], f32, tag=f"A{mt}", name=f"A{mt}") for mt in range(MT)]
            mm_proj(k_ps, "wk", "bk", xa_rhs)
            e2_t = stage.tile([128, MT, F], f32, tag="e", name="e")
            r2_t = stage.tile([128, MT, F], f32, tag="r", name="r")
            k_sb = stage.tile([128, MT, F], f32, tag="qk", name="qk")
            delu(k_sb, k_ps, e2_t, r2_t)

            # v' = gamma * esum * (wv x + bv)
            v_ps = [psum.tile([128, F], f32, tag=f"B{mt}", name=f"B{mt}") for mt in range(MT)]
            mm_proj(v_ps, "wv", "bv", xa_rhs)
            v_sb = stage.tile([128, MT, F], f32, tag="v", name="v")
            for mt in range(MT):
                nc.scalar.activation(
                    v_sb[:, mt, :], v_ps[mt], AF.Identity,
                    bias=0.0, scale=esg[:, mt:mt + 1],
                )

            # conv: d1 + conv3x3 + (bd1+bd3)
            cv_ps = [psum.tile([128, F], f32, tag=f"D{mt}", name=f"D{mt}") for mt in range(MT)]
            for mt in range(MT):
                msl = slice(mt * 128, (mt + 1) * 128)
                nc.tensor.matmul(
                    cv_ps[mt],
                    b_sb["bsum"][:, msl].bitcast(f32r),
                    ones_row.bitcast(f32r),
                    start=True,
                    stop=False,
                )
                first = False
                for tap in range(9):
                    ky, kx = tap // 3, tap % 3
                    for kt in range(KT):
                        nc.tensor.matmul(
                            cv_ps[mt],
                            wd3_sb[:, tap * KT + kt, msl].bitcast(f32r),
                            xa_rhs(kt, ky, kx),
                            start=False,
                            stop=False,
                        )
                for kt in range(KT):
                    nc.tensor.matmul(
                        cv_ps[mt],
                        w_sb["wd1"][:, kt, msl].bitcast(f32r),
                        xa_rhs(kt),
                        start=False,
                        stop=(kt == KT - 1),
                    )
                # wd1 matmuls close the accumulation group
            # kv = k * v'   (in place into k_sb's slot is avoided; own tile)
            kv = stage.tile([128, MT, F], f32, tag="kv", name="kv")
            nc.vector.tensor_mul(
                kv.rearrange("p t f -> p (t f)"),
                k_sb.rearrange("p t f -> p (t f)"),
                v_sb.rearrange("p t f -> p (t f)"),
            )
            # s3 = kv * norm_bcast  (in place)
            for mt in range(MT):
                nc.vector.tensor_mul(kv[:, mt, :], kv[:, mt, :], nb_ps)

            # c2 = gamma * conv + xa ; out = s3 + c2
            c2 = outp.tile([128, MT, CR, Wd], f32, tag="c2", name="c2")
            for mt in range(MT):
                nc.vector.scalar_tensor_tensor(
                    c2[:, mt],
                    cv_ps[mt].rearrange("p (a b) -> p a b", a=CR),
                    gam_col,
                    xar[:, mt],
                    op0=OP.mult,
                    op1=OP.add,
                )
            nc.gpsimd.tensor_add(
                c2.rearrange("p t a b -> p (t a b)"),
                c2.rearrange("p t a b -> p (t a b)"),
                kv.rearrange("p t f -> p (t f)"),
            )
            for mt in range(MT):
                nc.sync.dma_start(out=out_v[:, mt, r0:r0 + CR, :], in_=c2[:, mt])

    _fix_matmul_waits(nc)
    _fix_dma_waits(nc)
    nc.compile()
    return nc


def _fix_dma_waits(nc):
    """Cap DMACopy waits at 1 (walrus direct-DMA structs allow one).

    In-DMAs overwriting a rotating SBUF slot carry {compute release,
    DMA-queue WAW} waits. The compute release (readers of the previous
    generation finished) transitively implies the previous writer DMA
    finished (readers waited on it), so queue waits are dropped. Out-DMAs
    write disjoint DRAM rows (queue WAW vacuous) and their two c2 writers
    are ordered DVE -> Pool, so keep only the final writer's sem.
    """
    import concourse.mybir as mybir
    for name, inst in nc.inst_map.items():
        if not isinstance(inst, mybir.InstDMACopy):
            continue
        si = inst.sync_info
        if not si or not si.on_wait or len(si.on_wait) <= 1:
            continue
        comp = [w for w in si.on_wait
                if not str(w.ant_name).startswith(("DMAHW", "DMASW"))]
        if len(comp) == 0:
            si.on_wait = list(si.on_wait)[:1]
            continue
        si.on_wait = comp


def _fix_matmul_waits(nc):
    """Drop PE-self-semaphore waits from multi-wait PE instructions.

    walrus's S3_LW struct allows a single sync wait per (fused fp32r)
    matmul. Tile sometimes emits a PE-engine wait on PE's own completion
    semaphore alongside a cross-engine wait. PE matmuls start and complete
    strictly in program order (pc-monotone, per HW trace), so a PE
    instruction waiting on an earlier PE instruction's completion is
    redundant with queue order and safe to drop.
    """
    import concourse.mybir as mybir
    for name, inst in nc.inst_map.items():
        if not isinstance(inst, (mybir.InstMatmult, mybir.InstLdweights)):
            continue
        si = inst.sync_info
        if not si or not si.on_wait or len(si.on_wait) <= 1:
            continue
        kept = [w for w in si.on_wait if not str(w.ant_name).startswith("PE")]
        si.on_wait = kept if kept else list(si.on_wait)[:1]


def host_prep(x1, x2, wq1, bq1, wk1, bk1, wv1, bv1, wq2, bq2, wk2, bk2,
              wv2, bv2, wd1, bd1, wd3, bd3, gamma1, gamma2):
    """Build per-core input maps. Core i: batch i%4, stream i//4."""
    f = np.float32

    def wt(w):
        return np.ascontiguousarray(np.asarray(w, f).T)

    wd3T = np.ascontiguousarray(np.asarray(wd3, f).transpose(2, 3, 1, 0)
                                ).reshape(9, C, C)
    bsum = (np.asarray(bd1, f) + np.asarray(bd3, f)).reshape(1, C)
    streams = [
        dict(wqT=wt(wq1), wkT=wt(wk1), wvT=wt(wv1), wkoT=wt(wk2),
             bq=np.asarray(bq1, f).reshape(1, C), bk=np.asarray(bk1, f).reshape(1, C),
             bv=np.asarray(bv1, f).reshape(1, C), bko=np.asarray(bk2, f).reshape(1, C),
             gam=np.asarray(gamma1, f).reshape(1, 1)),
        dict(wqT=wt(wq2), wkT=wt(wk2), wvT=wt(wv2), wkoT=wt(wk1),
             bq=np.asarray(bq2, f).reshape(1, C), bk=np.asarray(bk2, f).reshape(1, C),
             bv=np.asarray(bv2, f).reshape(1, C), bko=np.asarray(bk1, f).reshape(1, C),
             gam=np.asarray(gamma2, f).reshape(1, 1)),
    ]
    shared = dict(wd1T=wt(wd1), wd3T=wd3T, bsum=bsum)
    xs = [np.asarray(x1, f), np.asarray(x2, f)]
    in_maps = []
    for core in range(N_CORES):
        b, s = core % B, core // B
        m = dict(shared)
        m.update(streams[s])
        m["xap"] = np.pad(xs[s][b], ((0, 0), (1, 1), (1, 1)))
        m["xb"] = np.ascontiguousarray(xs[1 - s][b])
        in_maps.append(m)
    return in_maps


_NC_CACHE = {}


def _get_nc():
    if "nc" not in _NC_CACHE:
        _NC_CACHE["nc"] = build_nc()
    return _NC_CACHE["nc"]


def run(in_maps, trace=False, **kw):
    from concourse.bass_utils import run_bass_kernel_spmd
    nc = _get_nc()
    return run_bass_kernel_spmd(nc, in_maps, core_ids=list(range(N_CORES)),
                                trace=trace, **kw)


def kernel(**inputs):
    in_maps = host_prep(**inputs)
    res = run(in_maps)
    out1 = np.stack([res.results[b]["out"] for b in range(B)])
    out2 = np.stack([res.results[B + b]["out"] for b in range(B)])
    return out1, out2
